# revision 4
# baseline (speedup 1.0000x reference)
"""TRN2 Bass kernel for nn_CLIP_DINOiser: data-parallel over batch (1 image/core).

Per core (one image):
  P = conv3x3(clip_feats, w_proj) + b_proj     [256, 2401]  (fp16 hi/lo 3-term split)
  P /= ||P||_c                                  (fp32)
  A = P^T P, thresholded at 0.2                [2401, 2401] (fp16 hi/lo 3-term, stored fp16)
  refined = (cm @ A) / (rowsum(A) + eps)       (fp16 matmuls; A symmetric so rows serve as cols)
  preds = (w_bkg . x) / ||x||_c + b_bkg        (fp16 hi only; smooth output)

Layouts: images padded to 51x51 (replicate, host-side) and flattened so every
matmul rhs is a contiguous slice; conv output columns x=49,50 are garbage and
skipped at the PSUM->SBUF extraction.
"""
import os
import sys

sys.path.insert(0, '/opt/trn_rl_repo')

import numpy as np

import concourse.bacc as bacc
import concourse.bass as bass
import concourse.tile as tile
from concourse import mybir
from concourse.bass_utils import run_bass_kernel_spmd

F32 = mybir.dt.float32
F16 = mybir.dt.float16
AF = mybir.ActivationFunctionType
OP = mybir.AluOpType

B = 8
H = W = 49
HW = H * W            # 2401
PW = 51               # padded width
PN = PW * PW + 3      # padded flat size (+3 so tap-shifted 510-wide reads stay in bounds)
CIN = 768
NCH = CIN // 128      # 6 input-channel chunks
COUT = 256
NMB = COUT // 128     # 2 projection blocks
CMC = 512             # clipmap channels
NCB = CMC // 128      # 4 clipmap blocks
MASK_TH = 0.2
EPS = 1e-6

CONV_TILES = [(0, 10), (10, 10), (20, 10), (30, 10), (40, 9)]   # (y0, rows)
COL_TILES = [(0, 512), (512, 512), (1024, 512), (1536, 512), (2048, 353)]
NB = (HW + 127) // 128   # 19 row blocks of A (last has 97 rows)


def _mbs(nb):
    return min(128, HW - nb * 128)


def _build():
    nc = bacc.Bacc("TRN2", target_bir_lowering=False, debug=False)

    xh_d = nc.declare_dram_parameter("xh", [NCH, 128, PN], F16, isOutput=False)
    xl_d = nc.declare_dram_parameter("xl", [NCH, 128, PN], F16, isOutput=False)
    wh_d = nc.declare_dram_parameter("wh", [9, NCH, 128, COUT], F16, isOutput=False)
    wl_d = nc.declare_dram_parameter("wl", [9, NCH, 128, COUT], F16, isOutput=False)
    cm_d = nc.declare_dram_parameter("cm16", [CMC, HW], F16, isOutput=False)
    bp_d = nc.declare_dram_parameter("bproj", [NMB, 128, 1], F32, isOutput=False)
    wb_d = nc.declare_dram_parameter("wb16", [NCH, 128, 1], F16, isOutput=False)
    bb_d = nc.declare_dram_parameter("bbkg", [1, 1], F32, isOutput=False)
    ref_d = nc.declare_dram_parameter("refined", [CMC, HW], F32, isOutput=True)
    prd_d = nc.declare_dram_parameter("preds", [1, HW], F32, isOutput=True)

    with tile.TileContext(nc) as tc:
        with tc.tile_pool(name="glob", bufs=1) as gp:
            ones16 = gp.tile([128, 1], F16, tag="ones16")
            nc.vector.memset(ones16, 1.0)
            ones32 = gp.tile([128, 1], F32, tag="ones32")
            nc.vector.memset(ones32, 1.0)
            ident16 = gp.tile([128, 128], F16, tag="ident16")
            nc.vector.memset(ident16, 0.0)
            nc.gpsimd.affine_select(
                out=ident16, in_=ident16, compare_op=OP.not_equal,
                fill=1.0, base=0, pattern=[[-1, 128]], channel_multiplier=1)
            bp_sb = gp.tile([128, NMB], F32, tag="bp")
            for mb in range(NMB):
                nc.sync.dma_start(out=bp_sb[:, mb:mb + 1], in_=bp_d[mb])
            wb_sb = gp.tile([128, NCH], F16, tag="wb")
            for c in range(NCH):
                nc.sync.dma_start(out=wb_sb[:, c:c + 1], in_=wb_d[c])
            bb_sb = gp.tile([1, 1], F32, tag="bb")
            nc.sync.dma_start(out=bb_sb, in_=bb_d[:, :])
            cmT = gp.tile([128, NB, CMC], F16, tag="cmT")

            # ---- Phase 1: transpose clipmap -> cmT[:, nb, c] ----
            with tc.tile_pool(name="ld", bufs=2) as ld, \
                 tc.tile_pool(name="ps1", bufs=2, space="PSUM") as ps1:
                for cb in range(NCB):
                    cm_sb = ld.tile([128, HW], F16, tag="cmld")
                    nc.sync.dma_start(out=cm_sb, in_=cm_d[cb * 128:(cb + 1) * 128, :])
                    for nb in range(NB):
                        m = _mbs(nb)
                        tp = ps1.tile([128, 128], F16, tag="trps")
                        nc.tensor.transpose(
                            tp[:m, :], cm_sb[:, nb * 128:nb * 128 + m], ident16)
                        nc.vector.tensor_copy(
                            out=cmT[:m, nb, cb * 128:(cb + 1) * 128], in_=tp[:m, :])

            with tc.tile_pool(name="pP", bufs=1) as pP:
                P = pP.tile([128, NMB, HW], F32, tag="P")

                # ---- Phase 2: conv + preds ----
                with tc.tile_pool(name="pX", bufs=1) as pX, \
                     tc.tile_pool(name="scrX", bufs=2) as sx:
                    xh = pX.tile([128, NCH, PN], F16, tag="xh")
                    xl = pX.tile([128, NCH, PN], F16, tag="xl")
                    for c in range(NCH):
                        nc.sync.dma_start(out=xh[:, c, :], in_=xh_d[c])
                        nc.sync.dma_start(out=xl[:, c, :], in_=xl_d[c])
                    wh = pX.tile([128, 9, NCH, COUT], F16, tag="wh")
                    wl = pX.tile([128, 9, NCH, COUT], F16, tag="wl")
                    for t in range(9):
                        for c in range(NCH):
                            nc.sync.dma_start(out=wh[:, t, c, :], in_=wh_d[t, c])
                            nc.sync.dma_start(out=wl[:, t, c, :], in_=wl_d[t, c])

                    with tc.tile_pool(name="ps2", bufs=2, space="PSUM") as ps2:
                        for (y0, rows) in CONV_TILES:
                            n = PW * rows
                            for mb in range(NMB):
                                ps = ps2.tile([128, 512], F32, tag="convps")
                                k, nmm = 0, 3 * 9 * NCH
                                for (wsb, xsb) in ((wh, xh), (wl, xh), (wh, xl)):
                                    for tap in range(9):
                                        dy, dx = tap // 3, tap % 3
                                        off = (y0 + dy) * PW + dx
                                        for c in range(NCH):
                                            nc.tensor.matmul(
                                                ps[:, :n],
                                                wsb[:, tap, c,
                                                    mb * 128:(mb + 1) * 128],
                                                xsb[:, c, off:off + n],
                                                start=(k == 0),
                                                stop=(k == nmm - 1))
                                            k += 1
                                src = ps[:, 0:n].rearrange(
                                    "p (r w) -> p r w", w=PW)[:, :, 0:W]
                                dst = P[:, mb, y0 * W:(y0 + rows) * W].rearrange(
                                    "p (r w) -> p r w", w=W)
                                nc.scalar.activation(
                                    out=dst, in_=src, func=AF.Identity,
                                    bias=bp_sb[:, mb:mb + 1], scale=1.0)

                    # preds: norm pass then dot pass (5 PSUM banks each, serial)
                    pbp = tc.tile_pool(name="pbp", bufs=1)
                    pbuf = pbp.__enter__().tile([1, 3, HW], F32, tag="pbuf")
                    with tc.tile_pool(name="psn", bufs=1, space="PSUM") as psn:
                        nsq_ps = psn.tile([1, 5, 512], F32, tag="nsqps")
                        for c in range(NCH):
                            x2 = sx.tile([128, PN], F16, tag="x2")
                            nc.scalar.activation(out=x2, in_=xh[:, c, :],
                                                 func=AF.Square)
                            for ti, (y0, rows) in enumerate(CONV_TILES):
                                n = PW * rows
                                off = (y0 + 1) * PW + 1
                                nc.tensor.matmul(
                                    nsq_ps[0:1, ti, :n], ones16,
                                    x2[:, off:off + n],
                                    start=(c == 0), stop=(c == NCH - 1))
                        for ti, (y0, rows) in enumerate(CONV_TILES):
                            src = nsq_ps[0:1, ti, 0:PW * rows].rearrange(
                                "p (r w) -> p r w", w=PW)[:, :, 0:W]
                            dst = pbuf[0:1, 0, y0 * W:(y0 + rows) * W].rearrange(
                                "p (r w) -> p r w", w=W)
                            nc.scalar.activation(out=dst, in_=src, func=AF.Sqrt)
                    with tc.tile_pool(name="psd", bufs=1, space="PSUM") as psd:
                        dot_ps = psd.tile([1, 5, 512], F32, tag="dotps")
                        for c in range(NCH):
                            for ti, (y0, rows) in enumerate(CONV_TILES):
                                n = PW * rows
                                off = (y0 + 1) * PW + 1
                                nc.tensor.matmul(
                                    dot_ps[0:1, ti, :n], wb_sb[:, c:c + 1],
                                    xh[:, c, off:off + n],
                                    start=(c == 0), stop=(c == NCH - 1))
                        for ti, (y0, rows) in enumerate(CONV_TILES):
                            src = dot_ps[0:1, ti, 0:PW * rows].rearrange(
                                "p (r w) -> p r w", w=PW)[:, :, 0:W]
                            dst = pbuf[0:1, 1, y0 * W:(y0 + rows) * W].rearrange(
                                "p (r w) -> p r w", w=W)
                            nc.vector.tensor_copy(out=dst, in_=src)
                    nc.vector.reciprocal(out=pbuf[0:1, 2, :], in_=pbuf[0:1, 0, :])
                    nc.vector.tensor_tensor(
                        out=pbuf[0:1, 1, :], in0=pbuf[0:1, 1, :],
                        in1=pbuf[0:1, 2, :], op=OP.mult)
                    nc.vector.tensor_scalar(
                        out=pbuf[0:1, 1, :], in0=pbuf[0:1, 1, :],
                        scalar1=bb_sb[0:1, 0:1], scalar2=None, op0=OP.add)
                    nc.sync.dma_start(out=prd_d[:, :], in_=pbuf[0:1, 1, :])
                    pbp.__exit__(None, None, None)

                # ---- Phase 3 + 4 ----
                with tc.tile_pool(name="pB", bufs=1) as pB:
                    Ph = pB.tile([128, NMB, HW], F16, tag="Ph")
                    Pl = pB.tile([128, NMB, HW], F16, tag="Pl")
                    Abig = pB.tile([128, NB, HW], F16, tag="Abig")

                    with tc.tile_pool(name="scr3", bufs=2) as s3, \
                         tc.tile_pool(name="sb2p", bufs=1) as s3b:
                        sbuf2 = s3b.tile([1, 2, HW], F32, tag="sbuf2")
                        with tc.tile_pool(name="ps3", bufs=1,
                                          space="PSUM") as ps3:
                            ssq_ps = ps3.tile([1, 5, 512], F32, tag="ssqps")
                            for mb in range(NMB):
                                for ti, (o, wd) in enumerate(COL_TILES):
                                    p2 = s3.tile([128, 512], F32, tag="p2")
                                    nc.scalar.activation(
                                        out=p2[:, :wd], in_=P[:, mb, o:o + wd],
                                        func=AF.Square)
                                    nc.tensor.matmul(
                                        ssq_ps[0:1, ti, :wd], ones32,
                                        p2[:, :wd],
                                        start=(mb == 0), stop=(mb == NMB - 1))
                            nc.scalar.activation(
                                out=sbuf2[0:1, 0, :],
                                in_=ssq_ps[0:1, :, :].rearrange(
                                    "p a b -> p (a b)")[:, 0:HW],
                                func=AF.Sqrt)
                        nc.vector.reciprocal(out=sbuf2[0:1, 1, :],
                                             in_=sbuf2[0:1, 0, :])
                        for ti, (o, wd) in enumerate(COL_TILES):
                            sbc = s3.tile([128, 512], F32, tag="sbc")
                            nc.gpsimd.partition_broadcast(
                                sbc[:, :wd], sbuf2[0:1, 1, o:o + wd])
                            for mb in range(NMB):
                                nc.vector.tensor_tensor(
                                    out=P[:, mb, o:o + wd],
                                    in0=P[:, mb, o:o + wd], in1=sbc[:, :wd],
                                    op=OP.mult)
                                nc.vector.tensor_copy(
                                    out=Ph[:, mb, o:o + wd],
                                    in_=P[:, mb, o:o + wd])
                                nc.vector.tensor_tensor(
                                    out=Pl[:, mb, o:o + wd],
                                    in0=P[:, mb, o:o + wd],
                                    in1=Ph[:, mb, o:o + wd], op=OP.subtract)

                    with tc.tile_pool(name="scr4", bufs=3) as s4, \
                         tc.tile_pool(name="ps4s", bufs=3, space="PSUM") as ps4s, \
                         tc.tile_pool(name="ps4r", bufs=2, space="PSUM") as ps4r, \
                         tc.tile_pool(name="ps4f", bufs=3, space="PSUM") as ps4f:
                        for ti, (o, wd) in enumerate(COL_TILES):
                            for nb in range(NB):
                                m = _mbs(nb)
                                sp = ps4s.tile([128, 512], F32, tag="sps")
                                k, nmm = 0, 3 * NMB
                                for (lh, rh) in ((Ph, Ph), (Pl, Ph), (Ph, Pl)):
                                    for c in range(NMB):
                                        nc.tensor.matmul(
                                            sp[:m, :wd],
                                            lh[:, c, nb * 128:nb * 128 + m],
                                            rh[:, c, o:o + wd],
                                            start=(k == 0), stop=(k == nmm - 1))
                                        k += 1
                                cs = s4.tile([128, 512], F32, tag="cs")
                                nc.scalar.activation(
                                    out=cs[:m, :wd], in_=sp[:m, :wd],
                                    func=AF.Identity)
                                nc.vector.scalar_tensor_tensor(
                                    out=Abig[:m, nb, o:o + wd],
                                    in0=cs[:m, :wd], scalar=MASK_TH,
                                    in1=cs[:m, :wd],
                                    op0=OP.is_ge, op1=OP.mult)
                            rp = ps4r.tile([1, 512], F32, tag="rps")
                            for nb in range(NB):
                                m = _mbs(nb)
                                nc.tensor.matmul(
                                    rp[0:1, :wd], ones16[:m, :],
                                    Abig[:m, nb, o:o + wd],
                                    start=(nb == 0), stop=(nb == NB - 1))
                            ri = s4.tile([1, 512], F32, tag="ri")
                            nc.vector.tensor_scalar(
                                out=ri[0:1, :wd], in0=rp[0:1, :wd],
                                scalar1=EPS, scalar2=None, op0=OP.add)
                            nc.vector.reciprocal(out=ri[0:1, :wd],
                                                 in_=ri[0:1, :wd])
                            rbc = s4.tile([128, 512], F32, tag="rbc")
                            nc.gpsimd.partition_broadcast(rbc[:, :wd],
                                                          ri[0:1, :wd])
                            for cb in range(NCB):
                                fp = ps4f.tile([128, 512], F32, tag="rfps")
                                for nb in range(NB):
                                    m = _mbs(nb)
                                    nc.tensor.matmul(
                                        fp[:, :wd],
                                        cmT[:m, nb, cb * 128:(cb + 1) * 128],
                                        Abig[:m, nb, o:o + wd],
                                        start=(nb == 0), stop=(nb == NB - 1))
                                ro = s4.tile([128, 512], F32, tag="ro")
                                nc.vector.tensor_tensor(
                                    out=ro[:, :wd], in0=fp[:, :wd],
                                    in1=rbc[:, :wd], op=OP.mult)
                                nc.sync.dma_start(
                                    out=ref_d[cb * 128:(cb + 1) * 128,
                                              o:o + wd],
                                    in_=ro[:, :wd])

    nc.finalize()
    return nc


_CACHE = {}


def _get_nc():
    if "nc" not in _CACHE:
        _CACHE["nc"] = _build()
    return _CACHE["nc"]


def _prep(clip_feats, clipmap, w_proj, b_proj, w_bkg, b_bkg):
    clip_feats = np.asarray(clip_feats, np.float32)
    clipmap = np.asarray(clipmap, np.float32)
    w_proj = np.asarray(w_proj, np.float32)
    b_proj = np.asarray(b_proj, np.float32).reshape(COUT)
    w_bkg = np.asarray(w_bkg, np.float32).reshape(CIN)
    b_bkg = np.asarray(b_bkg, np.float32).reshape(1)

    # [COUT, CIN, 3, 3] -> [9, NCH, 128, COUT] fp16 hi/lo
    wt = np.transpose(w_proj.reshape(COUT, NCH, 128, 9), (3, 1, 2, 0)).copy()
    wh = wt.astype(np.float16)
    wl = (wt - wh.astype(np.float32)).astype(np.float16)
    bp = np.ascontiguousarray(b_proj.reshape(NMB, 128, 1))
    wb = np.ascontiguousarray(w_bkg.reshape(NCH, 128, 1)).astype(np.float16)
    bb = b_bkg.reshape(1, 1)

    in_maps = []
    for b in range(B):
        xp = np.pad(clip_feats[b], ((0, 0), (1, 1), (1, 1)), mode='edge')
        xp = xp.reshape(CIN, PW * PW)
        xp = np.pad(xp, ((0, 0), (0, PN - PW * PW)))
        xh = xp.astype(np.float16)
        xl = (xp - xh.astype(np.float32)).astype(np.float16)
        in_maps.append({
            "xh": np.ascontiguousarray(xh.reshape(NCH, 128, PN)),
            "xl": np.ascontiguousarray(xl.reshape(NCH, 128, PN)),
            "wh": wh, "wl": wl,
            "cm16": clipmap[b].reshape(CMC, HW).astype(np.float16),
            "bproj": bp, "wb16": wb, "bbkg": bb,
        })
    return in_maps


def kernel(clip_feats, clipmap, w_proj, b_proj, w_bkg, b_bkg):
    nc = _get_nc()
    in_maps = _prep(clip_feats, clipmap, w_proj, b_proj, w_bkg, b_bkg)
    trace = bool(int(os.environ.get("KERNEL_TRACE", "0")))
    if trace:
        _install_trace_shim()
    res = run_bass_kernel_spmd(nc, in_maps, list(range(B)), trace=trace)
    _CACHE["last_exec_time_ns"] = res.exec_time_ns
    refined = np.stack([r["refined"] for r in res.results]).reshape(B, CMC, H, W)
    preds = np.stack([r["preds"] for r in res.results]).reshape(B, 1, H, W)
    return refined.astype(np.float32), preds.astype(np.float32)


def _install_trace_shim():
    import types
    if "antenv.axon_hooks" in sys.modules:
        return
    m = types.ModuleType('antenv.axon_hooks')
    hook = [None]
    m.set_axon_ntff_profile_hook = lambda h: hook.__setitem__(0, h)
    m.get_axon_ntff_profile_hook = lambda: hook[0]
    sys.modules['antenv.axon_hooks'] = m
    from trn_agent_boot.trn_boot import _ntff_profile_via_ctypes
    m.set_axon_ntff_profile_hook(
        _ntff_profile_via_ctypes('/opt/axon/libaxon_pjrt.so'))
    import concourse.bass_utils as bu
    bu.upload_artifacts = lambda tmpdir: "local://" + tmpdir


# revision 6
# speedup vs baseline: 1.0695x; 1.0695x over previous
"""TRN2 Bass kernel for nn_CLIP_DINOiser: data-parallel over batch (1 image/core).

Per core (one image):
  P = conv3x3(clip_feats, w_proj) + b_proj     [256, 2401]  (fp16 hi/lo 3-term split)
  P /= ||P||_c                                  (fp32)
  A = P^T P, thresholded at 0.2                [2401, 2401] (fp16 hi/lo 3-term, stored fp16)
  refined = (cm @ A) / (rowsum(A) + eps)       (fp16 matmuls; A symmetric so rows serve as cols)
  preds = (w_bkg . x) / ||x||_c + b_bkg        (fp16 hi only; smooth output)

Layouts: images padded to 51x51 (replicate, host-side) and flattened so every
matmul rhs is a contiguous slice; conv output columns x=49,50 are garbage and
skipped at the PSUM->SBUF extraction.
"""
import os
import sys

sys.path.insert(0, '/opt/trn_rl_repo')

import numpy as np

import concourse.bacc as bacc
import concourse.bass as bass
import concourse.tile as tile
from concourse import mybir
from concourse.bass_utils import run_bass_kernel_spmd

F32 = mybir.dt.float32
F16 = mybir.dt.float16
AF = mybir.ActivationFunctionType
OP = mybir.AluOpType

B = 8
H = W = 49
HW = H * W            # 2401
PW = 51               # padded width
PN = PW * PW + 3      # padded flat size (+3 so tap-shifted 510-wide reads stay in bounds)
CIN = 768
NCH = CIN // 128      # 6 input-channel chunks
COUT = 256
NMB = COUT // 128     # 2 projection blocks
CMC = 512             # clipmap channels
NCB = CMC // 128      # 4 clipmap blocks
MASK_TH = 0.2
EPS = 1e-6

CONV_TILES = [(0, 10), (10, 10), (20, 10), (30, 10), (40, 9)]   # (y0, rows)
COL_TILES = [(0, 512), (512, 512), (1024, 512), (1536, 512), (2048, 353)]
NB = (HW + 127) // 128   # 19 row blocks of A (last has 97 rows)


def _mbs(nb):
    return min(128, HW - nb * 128)


def _build():
    nc = bacc.Bacc("TRN2", target_bir_lowering=False, debug=False)

    xh_d = nc.declare_dram_parameter("xh", [NCH, 128, PN], F16, isOutput=False)
    xl_d = nc.declare_dram_parameter("xl", [NCH, 128, PN], F16, isOutput=False)
    wh_d = nc.declare_dram_parameter("wh", [9, NCH, 128, COUT], F16, isOutput=False)
    wl_d = nc.declare_dram_parameter("wl", [9, NCH, 128, COUT], F16, isOutput=False)
    cm_d = nc.declare_dram_parameter("cm16t", [NB, 128, CMC], F16, isOutput=False)
    bp_d = nc.declare_dram_parameter("bproj", [NMB, 128, 1], F32, isOutput=False)
    wb_d = nc.declare_dram_parameter("wb16", [NCH, 128, 1], F16, isOutput=False)
    bb_d = nc.declare_dram_parameter("bbkg", [1, 1], F32, isOutput=False)
    ref_d = nc.declare_dram_parameter("refined", [CMC, HW], F32, isOutput=True)
    prd_d = nc.declare_dram_parameter("preds", [1, HW], F32, isOutput=True)

    with tile.TileContext(nc) as tc:
        with tc.tile_pool(name="glob", bufs=1) as gp:
            ones16 = gp.tile([128, 1], F16, tag="ones16")
            nc.vector.memset(ones16, 1.0)
            ones32 = gp.tile([128, 1], F32, tag="ones32")
            nc.vector.memset(ones32, 1.0)
            bp_sb = gp.tile([128, NMB], F32, tag="bp")
            for mb in range(NMB):
                nc.sync.dma_start(out=bp_sb[:, mb:mb + 1], in_=bp_d[mb])
            wb_sb = gp.tile([128, NCH], F16, tag="wb")
            for c in range(NCH):
                nc.sync.dma_start(out=wb_sb[:, c:c + 1], in_=wb_d[c])
            bb_sb = gp.tile([1, 1], F32, tag="bb")
            nc.sync.dma_start(out=bb_sb, in_=bb_d[:, :])
            cmT = gp.tile([128, NB, CMC], F16, tag="cmT")
            sinv = gp.tile([1, HW], F32, tag="sinv")

            with tc.tile_pool(name="pP", bufs=1) as pP:
                P = pP.tile([128, NMB, HW], F32, tag="P")

                # ---- Phase 2: conv + preds ----
                with tc.tile_pool(name="pX", bufs=1) as pX, \
                     tc.tile_pool(name="scrX", bufs=2) as sx:
                    xh = pX.tile([128, NCH, PN], F16, tag="xh")
                    xl = pX.tile([128, NCH, PN], F16, tag="xl")
                    wh = pX.tile([128, 9, NCH, COUT], F16, tag="wh")
                    wl = pX.tile([128, 9, NCH, COUT], F16, tag="wl")
                    for c in range(NCH):
                        nc.sync.dma_start(out=xh[:, c, :], in_=xh_d[c])
                        for t in range(9):
                            nc.sync.dma_start(out=wh[:, t, c, :], in_=wh_d[t, c])
                    for c in range(NCH):
                        nc.sync.dma_start(out=xl[:, c, :], in_=xl_d[c])
                        for t in range(9):
                            nc.sync.dma_start(out=wl[:, t, c, :], in_=wl_d[t, c])
                    for nb in range(NB):
                        nc.sync.dma_start(out=cmT[:, nb, :], in_=cm_d[nb])

                    with tc.tile_pool(name="ps2", bufs=2, space="PSUM") as ps2:
                        for (y0, rows) in CONV_TILES:
                            n = PW * rows
                            for mb in range(NMB):
                                ps = ps2.tile([128, 512], F32, tag="convps")
                                k, nmm = 0, 3 * 9 * NCH
                                for (wsb, xsb) in ((wh, xh), (wl, xh), (wh, xl)):
                                    for tap in range(9):
                                        dy, dx = tap // 3, tap % 3
                                        off = (y0 + dy) * PW + dx
                                        for c in range(NCH):
                                            nc.tensor.matmul(
                                                ps[:, :n],
                                                wsb[:, tap, c,
                                                    mb * 128:(mb + 1) * 128],
                                                xsb[:, c, off:off + n],
                                                start=(k == 0),
                                                stop=(k == nmm - 1))
                                            k += 1
                                src = ps[:, 0:n].rearrange(
                                    "p (r w) -> p r w", w=PW)[:, :, 0:W]
                                dst = P[:, mb, y0 * W:(y0 + rows) * W].rearrange(
                                    "p (r w) -> p r w", w=W)
                                nc.scalar.activation(
                                    out=dst, in_=src, func=AF.Identity,
                                    bias=bp_sb[:, mb:mb + 1], scale=1.0)

                    # P column-norms: ssq colsums -> sinv = 1/||P||
                    with tc.tile_pool(name="ps3", bufs=1, space="PSUM") as ps3:
                        ssq_ps = ps3.tile([1, 5, 512], F32, tag="ssqps")
                        for mb in range(NMB):
                            for ti, (o, wd) in enumerate(COL_TILES):
                                p2 = sx.tile([128, 512], F32, tag="p2")
                                nc.scalar.activation(
                                    out=p2[:, :wd], in_=P[:, mb, o:o + wd],
                                    func=AF.Square)
                                nc.tensor.matmul(
                                    ssq_ps[0:1, ti, :wd], ones32, p2[:, :wd],
                                    start=(mb == 0), stop=(mb == NMB - 1))
                        nc.scalar.activation(
                            out=sinv[0:1, :],
                            in_=ssq_ps[0:1, :, :].rearrange(
                                "p a b -> p (a b)")[:, 0:HW],
                            func=AF.Sqrt)
                    nc.vector.reciprocal(out=sinv[0:1, :], in_=sinv[0:1, :])

                    # preds: norm pass then dot pass (5 PSUM banks each, serial)
                    pbp = tc.tile_pool(name="pbp", bufs=1)
                    pbuf = pbp.__enter__().tile([1, 3, HW], F32, tag="pbuf")
                    with tc.tile_pool(name="psn", bufs=1, space="PSUM") as psn:
                        nsq_ps = psn.tile([1, 5, 512], F32, tag="nsqps")
                        for c in range(NCH):
                            x2 = sx.tile([128, PN], F16, tag="x2")
                            nc.scalar.activation(out=x2, in_=xh[:, c, :],
                                                 func=AF.Square)
                            for ti, (y0, rows) in enumerate(CONV_TILES):
                                n = PW * rows
                                off = (y0 + 1) * PW + 1
                                nc.tensor.matmul(
                                    nsq_ps[0:1, ti, :n], ones16,
                                    x2[:, off:off + n],
                                    start=(c == 0), stop=(c == NCH - 1))
                        for ti, (y0, rows) in enumerate(CONV_TILES):
                            src = nsq_ps[0:1, ti, 0:PW * rows].rearrange(
                                "p (r w) -> p r w", w=PW)[:, :, 0:W]
                            dst = pbuf[0:1, 0, y0 * W:(y0 + rows) * W].rearrange(
                                "p (r w) -> p r w", w=W)
                            nc.scalar.activation(out=dst, in_=src, func=AF.Sqrt)
                    with tc.tile_pool(name="psd", bufs=1, space="PSUM") as psd:
                        dot_ps = psd.tile([1, 5, 512], F32, tag="dotps")
                        for c in range(NCH):
                            for ti, (y0, rows) in enumerate(CONV_TILES):
                                n = PW * rows
                                off = (y0 + 1) * PW + 1
                                nc.tensor.matmul(
                                    dot_ps[0:1, ti, :n], wb_sb[:, c:c + 1],
                                    xh[:, c, off:off + n],
                                    start=(c == 0), stop=(c == NCH - 1))
                        for ti, (y0, rows) in enumerate(CONV_TILES):
                            src = dot_ps[0:1, ti, 0:PW * rows].rearrange(
                                "p (r w) -> p r w", w=PW)[:, :, 0:W]
                            dst = pbuf[0:1, 1, y0 * W:(y0 + rows) * W].rearrange(
                                "p (r w) -> p r w", w=W)
                            nc.vector.tensor_copy(out=dst, in_=src)
                    nc.vector.reciprocal(out=pbuf[0:1, 2, :], in_=pbuf[0:1, 0, :])
                    nc.vector.tensor_tensor(
                        out=pbuf[0:1, 1, :], in0=pbuf[0:1, 1, :],
                        in1=pbuf[0:1, 2, :], op=OP.mult)
                    nc.vector.tensor_scalar(
                        out=pbuf[0:1, 1, :], in0=pbuf[0:1, 1, :],
                        scalar1=bb_sb[0:1, 0:1], scalar2=None, op0=OP.add)
                    nc.sync.dma_start(out=prd_d[:, :], in_=pbuf[0:1, 1, :])
                    pbp.__exit__(None, None, None)

                # ---- Phase 3 + 4 ----
                with tc.tile_pool(name="pB", bufs=1) as pB:
                    Ph = pB.tile([128, NMB, HW], F16, tag="Ph")
                    Pl = pB.tile([128, NMB, HW], F16, tag="Pl")
                    Abig = pB.tile([128, NB, HW], F16, tag="Abig")

                    with tc.tile_pool(name="scr3", bufs=2) as s3:
                        for ti, (o, wd) in enumerate(COL_TILES):
                            sbc = s3.tile([128, 512], F32, tag="sbc")
                            nc.gpsimd.partition_broadcast(
                                sbc[:, :wd], sinv[0:1, o:o + wd])
                            for mb in range(NMB):
                                nc.vector.tensor_tensor(
                                    out=P[:, mb, o:o + wd],
                                    in0=P[:, mb, o:o + wd], in1=sbc[:, :wd],
                                    op=OP.mult)
                                nc.vector.tensor_copy(
                                    out=Ph[:, mb, o:o + wd],
                                    in_=P[:, mb, o:o + wd])
                                nc.vector.tensor_tensor(
                                    out=Pl[:, mb, o:o + wd],
                                    in0=P[:, mb, o:o + wd],
                                    in1=Ph[:, mb, o:o + wd], op=OP.subtract)

                    with tc.tile_pool(name="scr4", bufs=3) as s4, \
                         tc.tile_pool(name="ps4s", bufs=3, space="PSUM") as ps4s, \
                         tc.tile_pool(name="ps4r", bufs=2, space="PSUM") as ps4r, \
                         tc.tile_pool(name="ps4f", bufs=3, space="PSUM") as ps4f:
                        for ti, (o, wd) in enumerate(COL_TILES):
                            for nb in range(NB):
                                m = _mbs(nb)
                                sp = ps4s.tile([128, 512], F32, tag="sps")
                                k, nmm = 0, 3 * NMB
                                for (lh, rh) in ((Ph, Ph), (Pl, Ph), (Ph, Pl)):
                                    for c in range(NMB):
                                        nc.tensor.matmul(
                                            sp[:m, :wd],
                                            lh[:, c, nb * 128:nb * 128 + m],
                                            rh[:, c, o:o + wd],
                                            start=(k == 0), stop=(k == nmm - 1))
                                        k += 1
                                cs = s4.tile([128, 512], F32, tag="cs")
                                nc.scalar.activation(
                                    out=cs[:m, :wd], in_=sp[:m, :wd],
                                    func=AF.Identity)
                                nc.vector.scalar_tensor_tensor(
                                    out=Abig[:m, nb, o:o + wd],
                                    in0=cs[:m, :wd], scalar=MASK_TH,
                                    in1=cs[:m, :wd],
                                    op0=OP.is_ge, op1=OP.mult)
                            rp = ps4r.tile([1, 512], F32, tag="rps")
                            for nb in range(NB):
                                m = _mbs(nb)
                                nc.tensor.matmul(
                                    rp[0:1, :wd], ones16[:m, :],
                                    Abig[:m, nb, o:o + wd],
                                    start=(nb == 0), stop=(nb == NB - 1))
                            ri = s4.tile([1, 512], F32, tag="ri")
                            nc.vector.tensor_scalar(
                                out=ri[0:1, :wd], in0=rp[0:1, :wd],
                                scalar1=EPS, scalar2=None, op0=OP.add)
                            nc.vector.reciprocal(out=ri[0:1, :wd],
                                                 in_=ri[0:1, :wd])
                            rbc = s4.tile([128, 512], F32, tag="rbc")
                            nc.gpsimd.partition_broadcast(rbc[:, :wd],
                                                          ri[0:1, :wd])
                            for cb in range(NCB):
                                fp = ps4f.tile([128, 512], F32, tag="rfps")
                                for nb in range(NB):
                                    m = _mbs(nb)
                                    nc.tensor.matmul(
                                        fp[:, :wd],
                                        cmT[:m, nb, cb * 128:(cb + 1) * 128],
                                        Abig[:m, nb, o:o + wd],
                                        start=(nb == 0), stop=(nb == NB - 1))
                                ro = s4.tile([128, 512], F32, tag="ro")
                                nc.vector.tensor_tensor(
                                    out=ro[:, :wd], in0=fp[:, :wd],
                                    in1=rbc[:, :wd], op=OP.mult)
                                nc.sync.dma_start(
                                    out=ref_d[cb * 128:(cb + 1) * 128,
                                              o:o + wd],
                                    in_=ro[:, :wd])

    nc.finalize()
    return nc


_CACHE = {}


def _get_nc():
    if "nc" not in _CACHE:
        _CACHE["nc"] = _build()
    return _CACHE["nc"]


def _prep(clip_feats, clipmap, w_proj, b_proj, w_bkg, b_bkg):
    clip_feats = np.asarray(clip_feats, np.float32)
    clipmap = np.asarray(clipmap, np.float32)
    w_proj = np.asarray(w_proj, np.float32)
    b_proj = np.asarray(b_proj, np.float32).reshape(COUT)
    w_bkg = np.asarray(w_bkg, np.float32).reshape(CIN)
    b_bkg = np.asarray(b_bkg, np.float32).reshape(1)

    # [COUT, CIN, 3, 3] -> [9, NCH, 128, COUT] fp16 hi/lo
    wt = np.transpose(w_proj.reshape(COUT, NCH, 128, 9), (3, 1, 2, 0)).copy()
    wh = wt.astype(np.float16)
    wl = (wt - wh.astype(np.float32)).astype(np.float16)
    bp = np.ascontiguousarray(b_proj.reshape(NMB, 128, 1))
    wb = np.ascontiguousarray(w_bkg.reshape(NCH, 128, 1)).astype(np.float16)
    bb = b_bkg.reshape(1, 1)

    in_maps = []
    for b in range(B):
        cmt = np.zeros((NB, 128, CMC), np.float16)
        cmf = clipmap[b].reshape(CMC, HW).T.astype(np.float16)  # [HW, CMC]
        for nb in range(NB):
            m = min(128, HW - nb * 128)
            cmt[nb, :m, :] = cmf[nb * 128:nb * 128 + m, :]
        xp = np.pad(clip_feats[b], ((0, 0), (1, 1), (1, 1)), mode='edge')
        xp = xp.reshape(CIN, PW * PW)
        xp = np.pad(xp, ((0, 0), (0, PN - PW * PW)))
        xh = xp.astype(np.float16)
        xl = (xp - xh.astype(np.float32)).astype(np.float16)
        in_maps.append({
            "xh": np.ascontiguousarray(xh.reshape(NCH, 128, PN)),
            "xl": np.ascontiguousarray(xl.reshape(NCH, 128, PN)),
            "wh": wh, "wl": wl,
            "cm16t": cmt,
            "bproj": bp, "wb16": wb, "bbkg": bb,
        })
    return in_maps


def kernel(clip_feats, clipmap, w_proj, b_proj, w_bkg, b_bkg):
    nc = _get_nc()
    in_maps = _prep(clip_feats, clipmap, w_proj, b_proj, w_bkg, b_bkg)
    trace = bool(int(os.environ.get("KERNEL_TRACE", "0")))
    if trace:
        _install_trace_shim()
    res = run_bass_kernel_spmd(nc, in_maps, list(range(B)), trace=trace)
    _CACHE["last_exec_time_ns"] = res.exec_time_ns
    refined = np.stack([r["refined"] for r in res.results]).reshape(B, CMC, H, W)
    preds = np.stack([r["preds"] for r in res.results]).reshape(B, 1, H, W)
    return refined.astype(np.float32), preds.astype(np.float32)


def _install_trace_shim():
    import types
    if "antenv.axon_hooks" in sys.modules:
        return
    m = types.ModuleType('antenv.axon_hooks')
    hook = [None]
    m.set_axon_ntff_profile_hook = lambda h: hook.__setitem__(0, h)
    m.get_axon_ntff_profile_hook = lambda: hook[0]
    sys.modules['antenv.axon_hooks'] = m
    from trn_agent_boot.trn_boot import _ntff_profile_via_ctypes
    m.set_axon_ntff_profile_hook(
        _ntff_profile_via_ctypes('/opt/axon/libaxon_pjrt.so'))
    import concourse.bass_utils as bu
    bu.upload_artifacts = lambda tmpdir: "local://" + tmpdir


# revision 9
# speedup vs baseline: 1.1114x; 1.0391x over previous
"""TRN2 Bass kernel for nn_CLIP_DINOiser: data-parallel over batch (1 image/core).

Per core (one image):
  P = conv3x3(clip_feats, w_proj) + b_proj     [256, 2401]  (fp16 hi/lo 3-term split)
  P /= ||P||_c                                  (fp32)
  A = P^T P, thresholded at 0.2                [2401, 2401] (fp16 hi/lo 3-term, stored fp16)
  refined = (cm @ A) / (rowsum(A) + eps)       (fp16 matmuls; A symmetric so rows serve as cols)
  preds = (w_bkg . x) / ||x||_c + b_bkg        (fp16 hi only; smooth output)

Layouts: images padded to 51x51 (replicate, host-side) and flattened so every
matmul rhs is a contiguous slice; conv output columns x=49,50 are garbage and
skipped at the PSUM->SBUF extraction.
"""
import os
import sys

sys.path.insert(0, '/opt/trn_rl_repo')

import numpy as np

import concourse.bacc as bacc
import concourse.bass as bass
import concourse.tile as tile
from concourse import mybir
from concourse.bass_utils import run_bass_kernel_spmd

F32 = mybir.dt.float32
F16 = mybir.dt.float16
AF = mybir.ActivationFunctionType
OP = mybir.AluOpType

B = 8
H = W = 49
HW = H * W            # 2401
PW = 51               # padded width
PN = PW * PW + 3      # padded flat size (+3 so tap-shifted 510-wide reads stay in bounds)
CIN = 768
NCH = CIN // 128      # 6 input-channel chunks
COUT = 256
NMB = COUT // 128     # 2 projection blocks
CMC = 512             # clipmap channels
NCB = CMC // 128      # 4 clipmap blocks
MASK_TH = 0.2
EPS = 1e-6

CONV_TILES = [(0, 10), (10, 10), (20, 10), (30, 10), (40, 9)]   # (y0, rows)
COL_TILES = [(0, 512), (512, 512), (1024, 512), (1536, 512), (2048, 353)]
NB = (HW + 127) // 128   # 19 row blocks of A (last has 97 rows)


def _mbs(nb):
    return min(128, HW - nb * 128)


def _build():
    nc = bacc.Bacc("TRN2", target_bir_lowering=False, debug=False)

    xh_d = nc.declare_dram_parameter("xh", [NCH, 128, PN], F16, isOutput=False)
    xl_d = nc.declare_dram_parameter("xl", [NCH, 128, PN], F16, isOutput=False)
    wh_d = nc.declare_dram_parameter("wh", [9, NCH, 128, COUT], F16, isOutput=False)
    wl_d = nc.declare_dram_parameter("wl", [9, NCH, 128, COUT], F16, isOutput=False)
    cm_d = nc.declare_dram_parameter("cm16t", [NB, 128, CMC], F16, isOutput=False)
    bp_d = nc.declare_dram_parameter("bproj", [NMB, 128, 1], F32, isOutput=False)
    wb_d = nc.declare_dram_parameter("wb16", [NCH, 128, 1], F16, isOutput=False)
    bb_d = nc.declare_dram_parameter("bbkg", [1, 1], F32, isOutput=False)
    ref_d = nc.declare_dram_parameter("refined", [CMC, HW], F32, isOutput=True)
    prd_d = nc.declare_dram_parameter("preds", [1, HW], F32, isOutput=True)

    with tile.TileContext(nc) as tc:
        with tc.tile_pool(name="glob", bufs=1) as gp:
            ones16 = gp.tile([128, 1], F16, tag="ones16")
            nc.vector.memset(ones16, 1.0)
            ones32 = gp.tile([128, 1], F32, tag="ones32")
            nc.vector.memset(ones32, 1.0)
            bp_sb = gp.tile([128, NMB], F32, tag="bp")
            for mb in range(NMB):
                nc.sync.dma_start(out=bp_sb[:, mb:mb + 1], in_=bp_d[mb])
            wb_sb = gp.tile([128, NCH], F16, tag="wb")
            for c in range(NCH):
                nc.sync.dma_start(out=wb_sb[:, c:c + 1], in_=wb_d[c])
            bb_sb = gp.tile([1, 1], F32, tag="bb")
            nc.sync.dma_start(out=bb_sb, in_=bb_d[:, :])
            cmT = gp.tile([128, NB, CMC], F16, tag="cmT")
            sinv = gp.tile([1, HW], F32, tag="sinv")

            with tc.tile_pool(name="pP", bufs=1) as pP:
                P = pP.tile([128, NMB, HW], F32, tag="P")

                # ---- Phase 2: conv + preds ----
                with tc.tile_pool(name="pX", bufs=1) as pX, \
                     tc.tile_pool(name="scrX", bufs=2) as sx:
                    xh = pX.tile([128, NCH, PN], F16, tag="xh")
                    xl = pX.tile([128, NCH, PN], F16, tag="xl")
                    wh = pX.tile([128, 9, NCH, COUT], F16, tag="wh")
                    wl = pX.tile([128, 9, NCH, COUT], F16, tag="wl")
                    for c in range(NCH):
                        nc.sync.dma_start(out=xh[:, c, :], in_=xh_d[c])
                    for c in range(NCH):
                        nc.sync.dma_start(
                            out=wh[:, :, c, :],
                            in_=wh_d[:, c].rearrange("t p o -> p t o"))
                    for c in range(NCH):
                        nc.sync.dma_start(
                            out=wl[:, :, c, :],
                            in_=wl_d[:, c].rearrange("t p o -> p t o"))
                    for c in range(NCH):
                        nc.sync.dma_start(out=xl[:, c, :], in_=xl_d[c])
                    nc.sync.dma_start(
                        out=cmT[:, :, :],
                        in_=cm_d.rearrange("n p c -> p n c"))

                    with tc.tile_pool(name="ps2", bufs=2, space="PSUM") as ps2:
                        for pi, (wsb, xsb) in enumerate(
                                ((wh, xh), (wl, xh), (wh, xl))):
                            for (y0, rows) in CONV_TILES:
                                n = PW * rows
                                for mb in range(NMB):
                                    ps = ps2.tile([128, 512], F32, tag="convps")
                                    k, nmm = 0, 9 * NCH
                                    for tap in range(9):
                                        dy, dx = tap // 3, tap % 3
                                        off = (y0 + dy) * PW + dx
                                        for c in range(NCH):
                                            nc.tensor.matmul(
                                                ps[:, :n],
                                                wsb[:, tap, c,
                                                    mb * 128:(mb + 1) * 128],
                                                xsb[:, c, off:off + n],
                                                start=(k == 0),
                                                stop=(k == nmm - 1))
                                            k += 1
                                    src = ps[:, 0:n].rearrange(
                                        "p (r w) -> p r w", w=PW)[:, :, 0:W]
                                    dst = P[:, mb,
                                            y0 * W:(y0 + rows) * W].rearrange(
                                        "p (r w) -> p r w", w=W)
                                    if pi == 0:
                                        nc.scalar.activation(
                                            out=dst, in_=src, func=AF.Identity,
                                            bias=bp_sb[:, mb:mb + 1], scale=1.0)
                                    else:
                                        nc.vector.tensor_tensor(
                                            out=dst, in0=src, in1=dst,
                                            op=OP.add)

                    # P column-norms: ssq colsums -> sinv = 1/||P||
                    with tc.tile_pool(name="ps3", bufs=1, space="PSUM") as ps3:
                        ssq_ps = ps3.tile([1, 5, 512], F32, tag="ssqps")
                        for mb in range(NMB):
                            for ti, (o, wd) in enumerate(COL_TILES):
                                p2 = sx.tile([128, 512], F32, tag="p2")
                                nc.scalar.activation(
                                    out=p2[:, :wd], in_=P[:, mb, o:o + wd],
                                    func=AF.Square)
                                nc.tensor.matmul(
                                    ssq_ps[0:1, ti, :wd], ones32, p2[:, :wd],
                                    start=(mb == 0), stop=(mb == NMB - 1))
                        nc.scalar.activation(
                            out=sinv[0:1, :],
                            in_=ssq_ps[0:1, :, :].rearrange(
                                "p a b -> p (a b)")[:, 0:HW],
                            func=AF.Sqrt)
                    nc.vector.reciprocal(out=sinv[0:1, :], in_=sinv[0:1, :])

                    # preds: norm pass then dot pass (5 PSUM banks each, serial)
                    pbp = tc.tile_pool(name="pbp", bufs=1)
                    pbuf = pbp.__enter__().tile([1, 3, HW], F32, tag="pbuf")
                    with tc.tile_pool(name="psn", bufs=1, space="PSUM") as psn:
                        nsq_ps = psn.tile([1, 5, 512], F32, tag="nsqps")
                        for c in range(NCH):
                            x2 = sx.tile([128, PN], F16, tag="x2")
                            nc.scalar.activation(out=x2, in_=xh[:, c, :],
                                                 func=AF.Square)
                            for ti, (y0, rows) in enumerate(CONV_TILES):
                                n = PW * rows
                                off = (y0 + 1) * PW + 1
                                nc.tensor.matmul(
                                    nsq_ps[0:1, ti, :n], ones16,
                                    x2[:, off:off + n],
                                    start=(c == 0), stop=(c == NCH - 1))
                        for ti, (y0, rows) in enumerate(CONV_TILES):
                            src = nsq_ps[0:1, ti, 0:PW * rows].rearrange(
                                "p (r w) -> p r w", w=PW)[:, :, 0:W]
                            dst = pbuf[0:1, 0, y0 * W:(y0 + rows) * W].rearrange(
                                "p (r w) -> p r w", w=W)
                            nc.scalar.activation(out=dst, in_=src, func=AF.Sqrt)
                    with tc.tile_pool(name="psd", bufs=1, space="PSUM") as psd:
                        dot_ps = psd.tile([1, 5, 512], F32, tag="dotps")
                        for c in range(NCH):
                            for ti, (y0, rows) in enumerate(CONV_TILES):
                                n = PW * rows
                                off = (y0 + 1) * PW + 1
                                nc.tensor.matmul(
                                    dot_ps[0:1, ti, :n], wb_sb[:, c:c + 1],
                                    xh[:, c, off:off + n],
                                    start=(c == 0), stop=(c == NCH - 1))
                        for ti, (y0, rows) in enumerate(CONV_TILES):
                            src = dot_ps[0:1, ti, 0:PW * rows].rearrange(
                                "p (r w) -> p r w", w=PW)[:, :, 0:W]
                            dst = pbuf[0:1, 1, y0 * W:(y0 + rows) * W].rearrange(
                                "p (r w) -> p r w", w=W)
                            nc.vector.tensor_copy(out=dst, in_=src)
                    nc.vector.reciprocal(out=pbuf[0:1, 2, :], in_=pbuf[0:1, 0, :])
                    nc.vector.tensor_tensor(
                        out=pbuf[0:1, 1, :], in0=pbuf[0:1, 1, :],
                        in1=pbuf[0:1, 2, :], op=OP.mult)
                    nc.vector.tensor_scalar(
                        out=pbuf[0:1, 1, :], in0=pbuf[0:1, 1, :],
                        scalar1=bb_sb[0:1, 0:1], scalar2=None, op0=OP.add)
                    nc.sync.dma_start(out=prd_d[:, :], in_=pbuf[0:1, 1, :])
                    pbp.__exit__(None, None, None)

                # ---- Phase 3 + 4 ----
                with tc.tile_pool(name="pB", bufs=1) as pB:
                    Ph_t = [pB.tile([128, NMB, 512], F16, tag=f"Ph{i}", name=f"Ph{i}")
                            for i in range(5)]
                    Pl_t = [pB.tile([128, NMB, 512], F16, tag=f"Pl{i}", name=f"Pl{i}")
                            for i in range(5)]
                    Abig = pB.tile([128, NB, HW], F16, tag="Abig")

                    with tc.tile_pool(name="scr3", bufs=2) as s3:
                        for ti, (o, wd) in enumerate(COL_TILES):
                            sbc = s3.tile([128, 512], F32, tag="sbc")
                            nc.gpsimd.partition_broadcast(
                                sbc[:, :wd], sinv[0:1, o:o + wd])
                            for mb in range(NMB):
                                nc.vector.tensor_tensor(
                                    out=P[:, mb, o:o + wd],
                                    in0=P[:, mb, o:o + wd], in1=sbc[:, :wd],
                                    op=OP.mult)
                                nc.vector.tensor_copy(
                                    out=Ph_t[ti][:, mb, :wd],
                                    in_=P[:, mb, o:o + wd])
                                nc.vector.tensor_tensor(
                                    out=Pl_t[ti][:, mb, :wd],
                                    in0=P[:, mb, o:o + wd],
                                    in1=Ph_t[ti][:, mb, :wd], op=OP.subtract)

                    with tc.tile_pool(name="scr4", bufs=3) as s4, \
                         tc.tile_pool(name="ps4s", bufs=3, space="PSUM") as ps4s, \
                         tc.tile_pool(name="ps4r", bufs=2, space="PSUM") as ps4r, \
                         tc.tile_pool(name="ps4f", bufs=3, space="PSUM") as ps4f:
                        for ti, (o, wd) in enumerate(COL_TILES):
                            for nb in range(NB):
                                m = _mbs(nb)
                                lti, loff = (nb * 128) // 512, (nb * 128) % 512
                                sp = ps4s.tile([128, 512], F32, tag="sps")
                                k, nmm = 0, 3 * NMB
                                for (lh, rh) in ((Ph_t, Ph_t), (Pl_t, Ph_t),
                                                 (Ph_t, Pl_t)):
                                    for c in range(NMB):
                                        nc.tensor.matmul(
                                            sp[:m, :wd],
                                            lh[lti][:, c, loff:loff + m],
                                            rh[ti][:, c, :wd],
                                            start=(k == 0), stop=(k == nmm - 1))
                                        k += 1
                                cs = s4.tile([128, 512], F32, tag="cs")
                                nc.scalar.activation(
                                    out=cs[:m, :wd], in_=sp[:m, :wd],
                                    func=AF.Identity)
                                nc.vector.scalar_tensor_tensor(
                                    out=Abig[:m, nb, o:o + wd],
                                    in0=cs[:m, :wd], scalar=MASK_TH,
                                    in1=cs[:m, :wd],
                                    op0=OP.is_ge, op1=OP.mult)
                            rp = ps4r.tile([1, 512], F32, tag="rps")
                            for nb in range(NB):
                                m = _mbs(nb)
                                nc.tensor.matmul(
                                    rp[0:1, :wd], ones16[:m, :],
                                    Abig[:m, nb, o:o + wd],
                                    start=(nb == 0), stop=(nb == NB - 1))
                            ri = s4.tile([1, 512], F32, tag="ri")
                            nc.vector.tensor_scalar(
                                out=ri[0:1, :wd], in0=rp[0:1, :wd],
                                scalar1=EPS, scalar2=None, op0=OP.add)
                            nc.vector.reciprocal(out=ri[0:1, :wd],
                                                 in_=ri[0:1, :wd])
                            rbc = s4.tile([128, 512], F32, tag="rbc")
                            nc.gpsimd.partition_broadcast(rbc[:, :wd],
                                                          ri[0:1, :wd])
                            for cb in range(NCB):
                                fp = ps4f.tile([128, 512], F32, tag="rfps")
                                for nb in range(NB):
                                    m = _mbs(nb)
                                    nc.tensor.matmul(
                                        fp[:, :wd],
                                        cmT[:m, nb, cb * 128:(cb + 1) * 128],
                                        Abig[:m, nb, o:o + wd],
                                        start=(nb == 0), stop=(nb == NB - 1))
                                ro = s4.tile([128, 512], F32, tag="ro")
                                nc.vector.tensor_tensor(
                                    out=ro[:, :wd], in0=fp[:, :wd],
                                    in1=rbc[:, :wd], op=OP.mult)
                                nc.sync.dma_start(
                                    out=ref_d[cb * 128:(cb + 1) * 128,
                                              o:o + wd],
                                    in_=ro[:, :wd])

    nc.finalize()
    return nc


_CACHE = {}


def _get_nc():
    if "nc" not in _CACHE:
        _CACHE["nc"] = _build()
    return _CACHE["nc"]


def _prep(clip_feats, clipmap, w_proj, b_proj, w_bkg, b_bkg):
    clip_feats = np.asarray(clip_feats, np.float32)
    clipmap = np.asarray(clipmap, np.float32)
    w_proj = np.asarray(w_proj, np.float32)
    b_proj = np.asarray(b_proj, np.float32).reshape(COUT)
    w_bkg = np.asarray(w_bkg, np.float32).reshape(CIN)
    b_bkg = np.asarray(b_bkg, np.float32).reshape(1)

    # [COUT, CIN, 3, 3] -> [9, NCH, 128, COUT] fp16 hi/lo
    wt = np.transpose(w_proj.reshape(COUT, NCH, 128, 9), (3, 1, 2, 0)).copy()
    wh = wt.astype(np.float16)
    wl = (wt - wh.astype(np.float32)).astype(np.float16)
    bp = np.ascontiguousarray(b_proj.reshape(NMB, 128, 1))
    wb = np.ascontiguousarray(w_bkg.reshape(NCH, 128, 1)).astype(np.float16)
    bb = b_bkg.reshape(1, 1)

    in_maps = []
    for b in range(B):
        cmt = np.zeros((NB, 128, CMC), np.float16)
        cmf = clipmap[b].reshape(CMC, HW).T.astype(np.float16)  # [HW, CMC]
        for nb in range(NB):
            m = min(128, HW - nb * 128)
            cmt[nb, :m, :] = cmf[nb * 128:nb * 128 + m, :]
        xp = np.pad(clip_feats[b], ((0, 0), (1, 1), (1, 1)), mode='edge')
        xp = xp.reshape(CIN, PW * PW)
        xp = np.pad(xp, ((0, 0), (0, PN - PW * PW)))
        xh = xp.astype(np.float16)
        xl = (xp - xh.astype(np.float32)).astype(np.float16)
        in_maps.append({
            "xh": np.ascontiguousarray(xh.reshape(NCH, 128, PN)),
            "xl": np.ascontiguousarray(xl.reshape(NCH, 128, PN)),
            "wh": wh, "wl": wl,
            "cm16t": cmt,
            "bproj": bp, "wb16": wb, "bbkg": bb,
        })
    return in_maps


def kernel(clip_feats, clipmap, w_proj, b_proj, w_bkg, b_bkg):
    nc = _get_nc()
    in_maps = _prep(clip_feats, clipmap, w_proj, b_proj, w_bkg, b_bkg)
    trace = bool(int(os.environ.get("KERNEL_TRACE", "0")))
    if trace:
        _install_trace_shim()
    res = run_bass_kernel_spmd(nc, in_maps, list(range(B)), trace=trace)
    _CACHE["last_exec_time_ns"] = res.exec_time_ns
    refined = np.stack([r["refined"] for r in res.results]).reshape(B, CMC, H, W)
    preds = np.stack([r["preds"] for r in res.results]).reshape(B, 1, H, W)
    return refined.astype(np.float32), preds.astype(np.float32)


def _install_trace_shim():
    import types
    if "antenv.axon_hooks" in sys.modules:
        return
    m = types.ModuleType('antenv.axon_hooks')
    hook = [None]
    m.set_axon_ntff_profile_hook = lambda h: hook.__setitem__(0, h)
    m.get_axon_ntff_profile_hook = lambda: hook[0]
    sys.modules['antenv.axon_hooks'] = m
    from trn_agent_boot.trn_boot import _ntff_profile_via_ctypes
    m.set_axon_ntff_profile_hook(
        _ntff_profile_via_ctypes('/opt/axon/libaxon_pjrt.so'))
    import concourse.bass_utils as bu
    bu.upload_artifacts = lambda tmpdir: "local://" + tmpdir
